# revision 16
# baseline (speedup 1.0000x reference)
"""Causal+padding-masked multi-head attention on 8 Trainium2 NeuronCores.

Problem: q[2,16,2048,64], k[2,16,64,2048], v[2,16,2048,64], mask_pad[2,1,1,2048]
-> out[2,16,2048,64] fp32 (softmax((q@k)/8 with pad+causal mask) @ v).

Sharding: batch*head data parallel - 32 (b,h) pairs, 4 per core; cores 0-3
take batch 0, cores 4-7 batch 1.

Per core, per (b,h) pair (all device matmuls in bf16, ~260ns per 512-wide):
  scoresT[t,s] = sum_d k[d,t]*(q[s,d]/8) + pad_bias[t]   (K=65: row 64 of kx
        holds pad_bias=-50 for masked keys, row 64 of qt is ones; the 1/8
        scale is folded into q on the host).
  Causal trimming: per 128-key chunk c only queries s >= 128c are computed.
        Score pieces (width 512/384/256/128) are packed into [128,1536] PSUM
        tiles; piece order keeps every matmul output inside one 2KB PSUM
        bank (a bank-crossing matmul silently loses its start/reset on the
        second bank and accumulates onto stale PSUM).
  exp: split between the Activation engine (one Exp instruction per score
        tile, ~1.0ns/col + 310ns) and the Vector engine for the 3 near-
        diagonal tiles per 512-query block j>=1 (Schraudolph in bf16 bit
        domain: int16(x*(2^7*log2 e) + (127*2^7 - c)) bitcast to bf16,
        ~3% max rel err, confined to <=50% of any query's weight mass).
  Diagonal 128x128 triangles are zeroed exactly AFTER exp by Vector-engine
        multiplies with a 0/1 tril mask (bf16 2x mode, cheaper than PE
        bias matmuls and keeps Schraudolph inputs >= -58).
  outT[d,s] = sum_c vxT[t,d] at[t,s] accumulated in [65,512] PSUM per
        query block; vx column 64 is ones so row 64 is the softmax
        denominator. outT is copied to SBUF and DMA'd out raw [65,2048].
Host: normalize out[s,d] = outT[d,s]/outT[64,s], transpose, and overwrite
  fully-padded prefix rows (all keys masked -> reference yields uniform
  weights) with mean(v) computed in numpy from mask_pad.
"""
import os
import sys

sys.path.insert(0, "/opt/trn_rl_repo")

import numpy as np

B, H, S, D = 2, 16, 2048, 64
NCORES = 8
BH_PER_CORE = (B * H) // NCORES  # 4
NCHUNK = S // 128   # 16 t-chunks of 128
NBLK = S // 512     # 4 s-blocks of 512
PAD_BIAS = -50.0    # post-scale pad bias on padded keys
TILE_W = 1536       # score tile width (3 PSUM banks)

# Schraudolph exp in the bf16 domain: bf16_bits(e^x) ~ int16(x*A + B)
EXP_A = float(np.float32(1.4426950408889634 * (1 << 7)))
EXP_B = float(np.float32(127.0 * (1 << 7) - 366393.0 / 65536.0))


def _build_schedule():
    """Per pair: list of score tiles. Each tile: dict(kind='ACT'|'DVE',
    pieces=[(j, c, s_lo, w, off, is_diag)], width). Emission order."""
    full_pieces = []   # (j, c, s_lo, w, diag=False)
    diag_tiles = {}    # j -> pieces
    for j in range(NBLK):
        for c in range(4 * j):
            full_pieces.append((j, c, 512 * j, 512, False))
        pieces = []
        for c in range(4 * j, 4 * j + 4):
            s_lo = 128 * c
            w = 512 * (j + 1) - s_lo
            pieces.append((j, c, s_lo, w, True))
        # order widths [512, 384, 128, 256] so no piece crosses a 512-element
        # PSUM bank boundary (a crossing matmul output silently loses its
        # start/reset on the second bank and accumulates onto stale PSUM)
        diag_tiles[j] = [pieces[0], pieces[1], pieces[3], pieces[2]]

    tiles = []

    def flush(buf, kind):
        if not buf:
            return
        off = 0
        pieces = []
        for (j, c, s_lo, w, dg) in buf:
            pieces.append((j, c, s_lo, w, off, dg))
            off += w
        tiles.append({"kind": kind, "pieces": pieces, "width": off})

    # j0 diag tile first (ACT for accuracy), then interleave fulls (3 per
    # tile) with each j's diag tile in j order.
    flush(diag_tiles[0], "ACT")
    buf = []
    fp = iter(full_pieces)
    fulls = list(fp)
    fi = 0
    for j in range(1, NBLK):
        # all full pieces of this j
        while fi < len(fulls) and fulls[fi][0] == j:
            buf.append(fulls[fi])
            fi += 1
            if len(buf) == 3:
                flush(buf, "ACT")
                buf = []
        flush(buf, "ACT")  # ragged remainder (keeps j-order for oT lifetime)
        buf = []
        flush(diag_tiles[j], "DVE")
    return tiles


SCHEDULE = _build_schedule()


def _register_ntff_shim():
    """The image's antenv lacks axon_hooks; register the NTFF profile hook so
    BASS_TRACE=1 works. Degrades silently if the axon boot pieces are absent."""
    import types
    if "antenv.axon_hooks" in sys.modules:
        return
    try:
        mod = types.ModuleType("antenv.axon_hooks")
        _hook = [None]
        mod.set_axon_ntff_profile_hook = lambda h: _hook.__setitem__(0, h)
        mod.get_axon_ntff_profile_hook = lambda: _hook[0]
        sys.modules["antenv.axon_hooks"] = mod
        import antenv
        antenv.axon_hooks = mod
        if "/root/.axon_site" not in sys.path:
            sys.path.insert(0, "/root/.axon_site")
        from trn_agent_boot.trn_boot import _ntff_profile_via_ctypes
        mod.set_axon_ntff_profile_hook(
            _ntff_profile_via_ctypes("/opt/axon/libaxon_pjrt.so"))
    except Exception:
        pass


def build_program():
    import concourse.bacc as bacc
    import concourse.tile as tile
    import concourse.mybir as mybir

    f32 = mybir.dt.float32
    f32r = mybir.dt.float32r
    bf16 = mybir.dt.bfloat16
    i32 = mybir.dt.int32
    AF = mybir.ActivationFunctionType
    ALU = mybir.AluOpType

    nc = bacc.Bacc("TRN2", target_bir_lowering=False, debug=False)

    qt_d = nc.dram_tensor("qt", [BH_PER_CORE, 65, S], bf16, kind="ExternalInput")
    kx_d = nc.dram_tensor("kx", [BH_PER_CORE, 65, S], bf16, kind="ExternalInput")
    vx_d = nc.dram_tensor("vx", [BH_PER_CORE, 128, NCHUNK, 65], bf16, kind="ExternalInput")
    msk_d = nc.dram_tensor("msk", [128, 128], bf16, kind="ExternalInput")
    ot_d = nc.dram_tensor("ot", [BH_PER_CORE, 65, S], f32, kind="ExternalOutput")
    DBG = os.environ.get("KDBG") == "1"
    if DBG:
        at_dbg = nc.dram_tensor("at_dbg", [len(SCHEDULE), 128, TILE_W], bf16,
                                kind="ExternalOutput")
        sc_dbg = nc.dram_tensor("sc_dbg", [len(SCHEDULE), 128, TILE_W], f32,
                                kind="ExternalOutput")

    with tile.TileContext(nc) as tc:
        with (
            tc.tile_pool(name="consts", bufs=1) as consts,
            tc.tile_pool(name="qt", bufs=BH_PER_CORE) as qt_pool,
            tc.tile_pool(name="kx", bufs=BH_PER_CORE) as kx_pool,
            tc.tile_pool(name="vx", bufs=BH_PER_CORE) as vx_pool,
            tc.tile_pool(name="at", bufs=3) as at_pool,
            tc.tile_pool(name="ye", bufs=2) as ye_pool,
            tc.tile_pool(name="ostage", bufs=2) as ostage_pool,
            tc.tile_pool(name="ps_sc", bufs=2, space="PSUM") as ps_sc,
            tc.tile_pool(name="ps_o", bufs=2, space="PSUM") as ps_o,
        ):
            # constants: [128,256] msk = [identity | trimask] in bf16
            warm_sb = consts.tile([128, 128], bf16)
            nc.gpsimd.memset(warm_sb[:].bitcast(mybir.dt.uint16), 0x3c00)
            msk_sb = consts.tile([128, 128], bf16)
            nc.sync.dma_start(msk_sb[:], msk_d[:, :])
            tril01 = msk_sb[:]  # 1.0 where s_loc >= t, else 0.0

            # PE warmup: flip the HAM clock gate to full speed (~4us) while
            # input DMAs are in flight.
            warm_ps = ps_sc.tile([128, TILE_W], f32, tag="sc")
            for w in range(32):
                nc.tensor.matmul(
                    warm_ps[:, 0:128], warm_sb[:], warm_sb[:],
                    start=(w == 0), stop=(w == 31), skip_group_check=True)

            deferred_av = None   # (l, tile_meta, at_t, oT_by_j)

            def emit_av(l, tmeta, at_t, oT_by_j):
                """AV matmuls for one score tile; returns list of finished j."""
                vx_sb = vx_by_l[l]
                done_j = []
                for (j, c, s_lo, w, off, dg) in tmeta["pieces"]:
                    oT = oT_by_j[j]
                    rel = s_lo - 512 * j
                    nc.tensor.matmul(
                        oT[:, rel:rel + w], vx_sb[:, c, :],
                        at_t[:, off:off + w],
                        start=(c == 0), stop=(c == 4 * j + 3),
                        skip_group_check=True)
                    if c == 4 * j + 3:
                        done_j.append(j)
                return done_j

            vx_by_l = {}
            ostage_by_l = {}

            def flush_out(pl, dj, poT):
                if pl < BH_PER_CORE - 1:
                    if pl not in ostage_by_l:
                        ostage_by_l[pl] = ostage_pool.tile(
                            [65, S], f32, tag="big", name=f"ostage{pl}")
                    ost = ostage_by_l[pl]
                    nc.vector.tensor_copy(
                        ost[:, 512 * dj:512 * (dj + 1)], poT[dj][:])
                    if dj == NBLK - 1:
                        nc.gpsimd.dma_start(ot_d[pl], ost[:])
                else:
                    # last pair: stream each j out as it completes to keep
                    # the tail short
                    ost = ostage_pool.tile([65, 512], f32, tag="small",
                                           name=f"ostj{dj}")
                    nc.vector.tensor_copy(ost[:], poT[dj][:])
                    nc.gpsimd.dma_start(
                        ot_d[pl, :, 512 * dj:512 * (dj + 1)], ost[:])

            for l in range(BH_PER_CORE):
                qt_sb = qt_pool.tile([65, S], bf16)
                kx_sb = kx_pool.tile([65, S], bf16)
                vx_sb = vx_pool.tile([128, NCHUNK, 65], bf16)
                nc.sync.dma_start(qt_sb[:], qt_d[l])
                nc.sync.dma_start(kx_sb[:], kx_d[l])
                nc.sync.dma_start(vx_sb[:], vx_d[l])
                vx_by_l[l] = vx_sb

                oT_by_j = {}

                for tmeta in SCHEDULE:
                    width = tmeta["width"]
                    sc = ps_sc.tile([128, TILE_W], f32, tag="sc")
                    # QK pieces (+ causal bias matmul on diagonal pieces)
                    for (j, c, s_lo, w, off, dg) in tmeta["pieces"]:
                        if j not in oT_by_j:
                            oT_by_j[j] = ps_o.tile(
                                [65, 512], f32, tag="oT", name=f"oT{l}_{j}")
                        nc.tensor.matmul(
                            sc[:, off:off + w],
                            kx_sb[:, 128 * c:128 * (c + 1)],
                            qt_sb[:, s_lo:s_lo + w],
                            start=True, stop=True, skip_group_check=True)
                    if DBG and l == 0:
                        ti_dbg = SCHEDULE.index(tmeta)
                        scc = ye_pool.tile([128, TILE_W], f32, tag="scc",
                                           name=f"scc{ti_dbg}")
                        nc.vector.tensor_copy(scc[:, 0:width], sc[:, 0:width])
                        nc.sync.dma_start(sc_dbg[ti_dbg], scc[:])
                    # exp
                    at_t = at_pool.tile([128, TILE_W], bf16, tag="at")
                    kind = tmeta["kind"]
                    if kind == "DVE" and l == BH_PER_CORE - 1 \
                            and tmeta is SCHEDULE[-1]:
                        kind = "ACT"  # shorter dependency chain on the tail
                    if kind == "ACT":
                        nc.scalar.activation(
                            at_t[:, 0:width], sc[:, 0:width], AF.Exp,
                            bias=0.0, scale=1.0)
                    else:
                        ye = ye_pool.tile([128, TILE_W], f32, tag="ye")
                        nc.vector.tensor_scalar(
                            ye[:, 0:width], sc[:, 0:width], EXP_A, EXP_B,
                            op0=ALU.mult, op1=ALU.add)
                        nc.vector.tensor_copy(
                            at_t[:, 0:width].bitcast(mybir.dt.int16),
                            ye[:, 0:width])
                    for (j, c, s_lo, w, off, dg) in tmeta["pieces"]:
                        if dg:
                            nc.vector.tensor_mul(
                                at_t[:, off:off + 128],
                                at_t[:, off:off + 128], tril01)
                    if DBG and l == 0:
                        nc.sync.dma_start(at_dbg[SCHEDULE.index(tmeta)], at_t[:])
                    # AV for previous tile
                    if deferred_av is not None:
                        pl, ptile, pat, poT = deferred_av
                        for dj in emit_av(pl, ptile, pat, poT):
                            flush_out(pl, dj, poT)
                    deferred_av = (l, tmeta, at_t, oT_by_j)

            # flush the last tile
            pl, ptile, pat, poT = deferred_av
            for dj in emit_av(pl, ptile, pat, poT):
                flush_out(pl, dj, poT)

    nc.compile()
    return nc


_PROGRAM = None
LAST_RESULTS = None


def kernel(q, k, v, mask_pad):
    global _PROGRAM, LAST_RESULTS
    import ml_dtypes
    bf = ml_dtypes.bfloat16

    q = np.asarray(q, dtype=np.float32)
    k = np.asarray(k, dtype=np.float32)
    v = np.asarray(v, dtype=np.float32)
    mask_pad = np.asarray(mask_pad)

    if os.environ.get("BASS_TRACE"):
        _register_ntff_shim()

    pad_bias = np.where(mask_pad[:, 0, 0, :] == 0,
                        np.float32(PAD_BIAS), np.float32(0.0))  # [B, S]

    # constant 0/1 lower-keep mask: keep at[t, s_loc] iff s_loc >= t
    tl, sl = np.meshgrid(np.arange(128), np.arange(128), indexing="ij")
    msk = np.where(sl >= tl, 1.0, 0.0).astype(np.float32).astype(bf)

    in_maps = []
    for core in range(NCORES):
        qt = np.empty((BH_PER_CORE, 65, S), np.float32)
        kx = np.empty((BH_PER_CORE, 65, S), np.float32)
        vx = np.empty((BH_PER_CORE, 128, NCHUNK, 65), np.float32)
        for l in range(BH_PER_CORE):
            bh = core * BH_PER_CORE + l
            b, h = bh // H, bh % H
            qt[l, :D] = q[b, h].T * np.float32(0.125)
            qt[l, D] = 1.0
            kx[l, :D] = k[b, h]
            kx[l, D] = pad_bias[b]
            vx[l, :, :, :D] = v[b, h].reshape(NCHUNK, 128, D).transpose(1, 0, 2)
            vx[l, :, :, D] = 1.0
        in_maps.append({"qt": qt.astype(bf), "kx": kx.astype(bf),
                        "vx": vx.astype(bf), "msk": msk})

    if _PROGRAM is None:
        _PROGRAM = build_program()

    from concourse.bass_utils import run_bass_kernel_spmd
    res = run_bass_kernel_spmd(_PROGRAM, in_maps, core_ids=list(range(NCORES)))
    LAST_RESULTS = res
    if res.exec_time_ns is not None:
        print(f"HW exec time: {res.exec_time_ns} ns")
        if res.profile_json:
            print(f"profile_json: {res.profile_json}")

    # host: normalize + transpose + fully-masked-row fixup
    out = np.empty((B, H, S, D), np.float32)
    first_one = np.zeros(B, dtype=np.int64)
    for b in range(B):
        nz = np.nonzero(mask_pad[b, 0, 0] != 0)[0]
        first_one[b] = nz[0] if len(nz) else S
    for core in range(NCORES):
        ot = res.results[core]["ot"]  # [BH_PER_CORE, 65, S]
        for l in range(BH_PER_CORE):
            bh = core * BH_PER_CORE + l
            b, h = bh // H, bh % H
            out[b, h] = (ot[l, :D] / ot[l, D]).T
            if first_one[b] > 0:
                out[b, h, :first_one[b]] = v[b, h].mean(axis=0)
    return out


# revision 17
# speedup vs baseline: 1.0122x; 1.0122x over previous
"""Causal+padding-masked multi-head attention on 8 Trainium2 NeuronCores.

Problem: q[2,16,2048,64], k[2,16,64,2048], v[2,16,2048,64], mask_pad[2,1,1,2048]
-> out[2,16,2048,64] fp32 (softmax((q@k)/8 with pad+causal mask) @ v).

Sharding: batch*head data parallel - 32 (b,h) pairs, 4 per core; cores 0-3
take batch 0, cores 4-7 batch 1.

Per core, per (b,h) pair (all device matmuls in bf16, ~260ns per 512-wide):
  scoresT[t,s] = sum_d k[d,t]*(q[s,d]/8) + pad_bias[t]   (K=65: row 64 of kx
        holds pad_bias=-50 for masked keys, row 64 of qt is ones; the 1/8
        scale is folded into q on the host).
  Causal trimming: per 128-key chunk c only queries s >= 128c are computed.
        Score pieces (width 512/384/256/128) are packed into [128,1536] PSUM
        tiles; piece order keeps every matmul output inside one 2KB PSUM
        bank (a bank-crossing matmul silently loses its start/reset on the
        second bank and accumulates onto stale PSUM).
  exp: split between the Activation engine (one Exp instruction per score
        tile, ~1.0ns/col + 310ns) and the Vector engine for the 3 near-
        diagonal tiles per 512-query block j>=1 (Schraudolph in bf16 bit
        domain: int16(x*(2^7*log2 e) + (127*2^7 - c)) bitcast to bf16,
        ~3% max rel err, confined to <=50% of any query's weight mass).
  Diagonal 128x128 triangles are zeroed exactly AFTER exp by Vector-engine
        multiplies with a 0/1 tril mask (bf16 2x mode, cheaper than PE
        bias matmuls and keeps Schraudolph inputs >= -58).
  outT[d,s] = sum_c vxT[t,d] at[t,s] accumulated in [65,512] PSUM per
        query block; vx column 64 is ones so row 64 is the softmax
        denominator. outT is copied to SBUF and DMA'd out raw [65,2048].
Host: normalize out[s,d] = outT[d,s]/outT[64,s], transpose, and overwrite
  fully-padded prefix rows (all keys masked -> reference yields uniform
  weights) with mean(v) computed in numpy from mask_pad.
"""
import os
import sys

sys.path.insert(0, "/opt/trn_rl_repo")

import numpy as np

B, H, S, D = 2, 16, 2048, 64
NCORES = 8
BH_PER_CORE = (B * H) // NCORES  # 4
NCHUNK = S // 128   # 16 t-chunks of 128
NBLK = S // 512     # 4 s-blocks of 512
PAD_BIAS = -50.0    # post-scale pad bias on padded keys
TILE_W = 1536       # score tile width (3 PSUM banks)

# Schraudolph exp in the bf16 domain: bf16_bits(e^x) ~ int16(x*A + B)
EXP_A = float(np.float32(1.4426950408889634 * (1 << 7)))
EXP_B = float(np.float32(127.0 * (1 << 7) - 366393.0 / 65536.0))


def _build_schedule():
    """Per pair: list of score tiles. Each tile: dict(kind='ACT'|'DVE',
    pieces=[(j, c, s_lo, w, off, is_diag)], width). Emission order."""
    full_pieces = []   # (j, c, s_lo, w, diag=False)
    diag_tiles = {}    # j -> pieces
    for j in range(NBLK):
        for c in range(4 * j):
            full_pieces.append((j, c, 512 * j, 512, False))
        pieces = []
        for c in range(4 * j, 4 * j + 4):
            s_lo = 128 * c
            w = 512 * (j + 1) - s_lo
            pieces.append((j, c, s_lo, w, True))
        # order widths [512, 384, 128, 256] so no piece crosses a 512-element
        # PSUM bank boundary (a crossing matmul output silently loses its
        # start/reset on the second bank and accumulates onto stale PSUM)
        diag_tiles[j] = [pieces[0], pieces[1], pieces[3], pieces[2]]

    tiles = []

    def flush(buf, kind):
        if not buf:
            return
        off = 0
        pieces = []
        for (j, c, s_lo, w, dg) in buf:
            pieces.append((j, c, s_lo, w, off, dg))
            off += w
        tiles.append({"kind": kind, "pieces": pieces, "width": off})

    # j0 diag tile first (ACT for accuracy), then interleave fulls (3 per
    # tile) with each j's diag tile in j order.
    flush(diag_tiles[0], "ACT")
    buf = []
    fp = iter(full_pieces)
    fulls = list(fp)
    fi = 0
    for j in range(1, NBLK):
        # all full pieces of this j
        while fi < len(fulls) and fulls[fi][0] == j:
            buf.append(fulls[fi])
            fi += 1
            if len(buf) == 3:
                flush(buf, "ACT")
                buf = []
        flush(buf, "ACT")  # ragged remainder (keeps j-order for oT lifetime)
        buf = []
        flush(diag_tiles[j], "DVE")
    return tiles


SCHEDULE = _build_schedule()


def _register_ntff_shim():
    """The image's antenv lacks axon_hooks; register the NTFF profile hook so
    BASS_TRACE=1 works. Degrades silently if the axon boot pieces are absent."""
    import types
    if "antenv.axon_hooks" in sys.modules:
        return
    try:
        mod = types.ModuleType("antenv.axon_hooks")
        _hook = [None]
        mod.set_axon_ntff_profile_hook = lambda h: _hook.__setitem__(0, h)
        mod.get_axon_ntff_profile_hook = lambda: _hook[0]
        sys.modules["antenv.axon_hooks"] = mod
        import antenv
        antenv.axon_hooks = mod
        if "/root/.axon_site" not in sys.path:
            sys.path.insert(0, "/root/.axon_site")
        from trn_agent_boot.trn_boot import _ntff_profile_via_ctypes
        mod.set_axon_ntff_profile_hook(
            _ntff_profile_via_ctypes("/opt/axon/libaxon_pjrt.so"))
    except Exception:
        pass


def build_program():
    import concourse.bacc as bacc
    import concourse.tile as tile
    import concourse.mybir as mybir

    f32 = mybir.dt.float32
    f32r = mybir.dt.float32r
    bf16 = mybir.dt.bfloat16
    i32 = mybir.dt.int32
    AF = mybir.ActivationFunctionType
    ALU = mybir.AluOpType

    nc = bacc.Bacc("TRN2", target_bir_lowering=False, debug=False)

    qt_d = nc.dram_tensor("qt", [BH_PER_CORE, 65, S], bf16, kind="ExternalInput")
    kx_d = nc.dram_tensor("kx", [BH_PER_CORE, 65, S], bf16, kind="ExternalInput")
    vx_d = nc.dram_tensor("vx", [BH_PER_CORE, 128, NCHUNK, 65], bf16, kind="ExternalInput")
    msk_d = nc.dram_tensor("msk", [128, 128], bf16, kind="ExternalInput")
    ot_d = nc.dram_tensor("ot", [BH_PER_CORE, 65, S], f32, kind="ExternalOutput")
    DBG = os.environ.get("KDBG") == "1"
    if DBG:
        at_dbg = nc.dram_tensor("at_dbg", [len(SCHEDULE), 128, TILE_W], bf16,
                                kind="ExternalOutput")
        sc_dbg = nc.dram_tensor("sc_dbg", [len(SCHEDULE), 128, TILE_W], f32,
                                kind="ExternalOutput")

    with tile.TileContext(nc) as tc:
        with (
            tc.tile_pool(name="consts", bufs=1) as consts,
            tc.tile_pool(name="qt", bufs=BH_PER_CORE) as qt_pool,
            tc.tile_pool(name="kx", bufs=BH_PER_CORE) as kx_pool,
            tc.tile_pool(name="vx", bufs=BH_PER_CORE) as vx_pool,
            tc.tile_pool(name="at", bufs=4) as at_pool,
            tc.tile_pool(name="ye", bufs=3) as ye_pool,
            tc.tile_pool(name="ostage", bufs=2) as ostage_pool,
            tc.tile_pool(name="ps_sc", bufs=2, space="PSUM") as ps_sc,
            tc.tile_pool(name="ps_o", bufs=2, space="PSUM") as ps_o,
        ):
            # constants: [128,256] msk = [identity | trimask] in bf16
            warm_sb = consts.tile([128, 128], bf16)
            nc.gpsimd.memset(warm_sb[:].bitcast(mybir.dt.uint16), 0x3c00)
            msk_sb = consts.tile([128, 128], bf16)
            nc.sync.dma_start(msk_sb[:], msk_d[:, :])
            tril01 = msk_sb[:]  # 1.0 where s_loc >= t, else 0.0

            # PE warmup: flip the HAM clock gate to full speed (~4us) while
            # input DMAs are in flight.
            warm_ps = ps_sc.tile([128, TILE_W], f32, tag="sc")
            for w in range(32):
                nc.tensor.matmul(
                    warm_ps[:, 0:128], warm_sb[:], warm_sb[:],
                    start=(w == 0), stop=(w == 31), skip_group_check=True)

            deferred_av = None   # (l, tile_meta, at_t, oT_by_j)

            def emit_av(l, tmeta, at_t, oT_by_j):
                """AV matmuls for one score tile; returns list of finished j."""
                vx_sb = vx_by_l[l]
                done_j = []
                for (j, c, s_lo, w, off, dg) in tmeta["pieces"]:
                    oT = oT_by_j[j]
                    rel = s_lo - 512 * j
                    nc.tensor.matmul(
                        oT[:, rel:rel + w], vx_sb[:, c, :],
                        at_t[:, off:off + w],
                        start=(c == 0), stop=(c == 4 * j + 3),
                        skip_group_check=True)
                    if c == 4 * j + 3:
                        done_j.append(j)
                return done_j

            vx_by_l = {}
            ostage_by_l = {}

            def flush_out(pl, dj, poT):
                if pl < BH_PER_CORE - 1:
                    if pl not in ostage_by_l:
                        ostage_by_l[pl] = ostage_pool.tile(
                            [65, S], f32, tag="big", name=f"ostage{pl}")
                    ost = ostage_by_l[pl]
                    nc.vector.tensor_copy(
                        ost[:, 512 * dj:512 * (dj + 1)], poT[dj][:])
                    if dj == NBLK - 1:
                        nc.gpsimd.dma_start(ot_d[pl], ost[:])
                else:
                    # last pair: stream each j out as it completes to keep
                    # the tail short
                    ost = ostage_pool.tile([65, 512], f32, tag="small",
                                           name=f"ostj{dj}")
                    nc.vector.tensor_copy(ost[:], poT[dj][:])
                    nc.gpsimd.dma_start(
                        ot_d[pl, :, 512 * dj:512 * (dj + 1)], ost[:])

            for l in range(BH_PER_CORE):
                qt_sb = qt_pool.tile([65, S], bf16)
                kx_sb = kx_pool.tile([65, S], bf16)
                vx_sb = vx_pool.tile([128, NCHUNK, 65], bf16)
                nc.sync.dma_start(qt_sb[:], qt_d[l])
                nc.sync.dma_start(kx_sb[:], kx_d[l])
                nc.sync.dma_start(vx_sb[:], vx_d[l])
                vx_by_l[l] = vx_sb

                oT_by_j = {}

                for tmeta in SCHEDULE:
                    width = tmeta["width"]
                    sc = ps_sc.tile([128, TILE_W], f32, tag="sc")
                    # QK pieces (+ causal bias matmul on diagonal pieces)
                    for (j, c, s_lo, w, off, dg) in tmeta["pieces"]:
                        if j not in oT_by_j:
                            oT_by_j[j] = ps_o.tile(
                                [65, 512], f32, tag="oT", name=f"oT{l}_{j}")
                        nc.tensor.matmul(
                            sc[:, off:off + w],
                            kx_sb[:, 128 * c:128 * (c + 1)],
                            qt_sb[:, s_lo:s_lo + w],
                            start=True, stop=True, skip_group_check=True)
                    if DBG and l == 0:
                        ti_dbg = SCHEDULE.index(tmeta)
                        scc = ye_pool.tile([128, TILE_W], f32, tag="scc",
                                           name=f"scc{ti_dbg}")
                        nc.vector.tensor_copy(scc[:, 0:width], sc[:, 0:width])
                        nc.sync.dma_start(sc_dbg[ti_dbg], scc[:])
                    # exp
                    at_t = at_pool.tile([128, TILE_W], bf16, tag="at")
                    kind = tmeta["kind"]
                    if kind == "DVE" and l == BH_PER_CORE - 1 \
                            and tmeta is SCHEDULE[-1]:
                        kind = "ACT"  # shorter dependency chain on the tail
                    if kind == "ACT":
                        nc.scalar.activation(
                            at_t[:, 0:width], sc[:, 0:width], AF.Exp,
                            bias=0.0, scale=1.0)
                    else:
                        ye = ye_pool.tile([128, TILE_W], f32, tag="ye")
                        nc.vector.tensor_scalar(
                            ye[:, 0:width], sc[:, 0:width], EXP_A, EXP_B,
                            op0=ALU.mult, op1=ALU.add)
                        nc.vector.tensor_copy(
                            at_t[:, 0:width].bitcast(mybir.dt.int16),
                            ye[:, 0:width])
                    for (j, c, s_lo, w, off, dg) in tmeta["pieces"]:
                        if dg:
                            nc.vector.tensor_mul(
                                at_t[:, off:off + 128],
                                at_t[:, off:off + 128], tril01)
                    if DBG and l == 0:
                        nc.sync.dma_start(at_dbg[SCHEDULE.index(tmeta)], at_t[:])
                    # AV for previous tile
                    if deferred_av is not None:
                        pl, ptile, pat, poT = deferred_av
                        for dj in emit_av(pl, ptile, pat, poT):
                            flush_out(pl, dj, poT)
                    deferred_av = (l, tmeta, at_t, oT_by_j)

            # flush the last tile
            pl, ptile, pat, poT = deferred_av
            for dj in emit_av(pl, ptile, pat, poT):
                flush_out(pl, dj, poT)

    nc.compile()
    return nc


_PROGRAM = None
LAST_RESULTS = None


def kernel(q, k, v, mask_pad):
    global _PROGRAM, LAST_RESULTS
    import ml_dtypes
    bf = ml_dtypes.bfloat16

    q = np.asarray(q, dtype=np.float32)
    k = np.asarray(k, dtype=np.float32)
    v = np.asarray(v, dtype=np.float32)
    mask_pad = np.asarray(mask_pad)

    if os.environ.get("BASS_TRACE"):
        _register_ntff_shim()

    pad_bias = np.where(mask_pad[:, 0, 0, :] == 0,
                        np.float32(PAD_BIAS), np.float32(0.0))  # [B, S]

    # constant 0/1 lower-keep mask: keep at[t, s_loc] iff s_loc >= t
    tl, sl = np.meshgrid(np.arange(128), np.arange(128), indexing="ij")
    msk = np.where(sl >= tl, 1.0, 0.0).astype(np.float32).astype(bf)

    in_maps = []
    for core in range(NCORES):
        qt = np.empty((BH_PER_CORE, 65, S), np.float32)
        kx = np.empty((BH_PER_CORE, 65, S), np.float32)
        vx = np.empty((BH_PER_CORE, 128, NCHUNK, 65), np.float32)
        for l in range(BH_PER_CORE):
            bh = core * BH_PER_CORE + l
            b, h = bh // H, bh % H
            qt[l, :D] = q[b, h].T * np.float32(0.125)
            qt[l, D] = 1.0
            kx[l, :D] = k[b, h]
            kx[l, D] = pad_bias[b]
            vx[l, :, :, :D] = v[b, h].reshape(NCHUNK, 128, D).transpose(1, 0, 2)
            vx[l, :, :, D] = 1.0
        in_maps.append({"qt": qt.astype(bf), "kx": kx.astype(bf),
                        "vx": vx.astype(bf), "msk": msk})

    if _PROGRAM is None:
        _PROGRAM = build_program()

    from concourse.bass_utils import run_bass_kernel_spmd
    res = run_bass_kernel_spmd(_PROGRAM, in_maps, core_ids=list(range(NCORES)))
    LAST_RESULTS = res
    if res.exec_time_ns is not None:
        print(f"HW exec time: {res.exec_time_ns} ns")
        if res.profile_json:
            print(f"profile_json: {res.profile_json}")

    # host: normalize + transpose + fully-masked-row fixup
    out = np.empty((B, H, S, D), np.float32)
    first_one = np.zeros(B, dtype=np.int64)
    for b in range(B):
        nz = np.nonzero(mask_pad[b, 0, 0] != 0)[0]
        first_one[b] = nz[0] if len(nz) else S
    for core in range(NCORES):
        ot = res.results[core]["ot"]  # [BH_PER_CORE, 65, S]
        for l in range(BH_PER_CORE):
            bh = core * BH_PER_CORE + l
            b, h = bh // H, bh % H
            out[b, h] = (ot[l, :D] / ot[l, D]).T
            if first_one[b] > 0:
                out[b, h, :first_one[b]] = v[b, h].mean(axis=0)
    return out
